# revision 8
# baseline (speedup 1.0000x reference)
"""Cross-attention Trainium2 kernel (8 NeuronCores, SPMD).

Reference computation (per full batch):
  q = x @ Wq + bq;  k = enc @ Wk (bk dropped: softmax-invariant);
  v = enc @ Wv + bv
  att = softmax((q k^T) / sqrt(D));  y = (att v) @ Wo + bo

Sharding: B(=4) x T-half(=2) -> 8 cores. Each core handles one batch
element and half of the 2048 query tokens, with all 16 heads, and
produces out[b, t_half] directly (host only reassembles).

Design (v3; ~20% faster than the f32r baseline on HW):
  - Everything bf16 on the matmul paths; the host pre-casts x/enc and
    the four weight matrices to bf16 (halves DMA bytes and SBUF).
    End-to-end rel err ~3e-3 (gate is 2e-2).
  - bk is dropped entirely: k-bias adds q.bk, constant across kv
    positions, so softmax is invariant to it.
  - K=64 scores matmuls taken straight from kT row halves; the two
    heads of a 128-row chunk issue back-to-back so their row-disjoint
    matmuls run concurrently in the PE array (measured ~1.9x).
  - V is stored pre-padded as 65-col blocks [v_h + bv | 1] per head
    (vS), so each AV matmul's lhsT slice yields y rows 0..63 and the
    softmax denominator in row 64 for free; built during the V-proj
    PSUM->SBUF copy with strided APs. bv baked in => normalization
    directly produces y + bv.
  - Normalization: reciprocal of the denominator row, GPSIMD partition
    broadcast, one DVE multiply -> yT (bf16).
  - One shared 4-buffer PSUM ring ([128,512] tiles, 4 banks) carries
    transposes, all projection chunks and the score tiles; the 4 ya
    accumulators take the other 4 banks. The deep ring keeps the
    score->exp->AV chain pipelined (exp-wait WARs resolve a full wave
    early); K/Q projection chunks for the next head pair are emitted as
    fine-grained 2-matmul steps that fill the exp-wait slack.
  - Per-sc emission is hand-ordered for the strict in-order engine
    queues: paired scores, then deferred AV matmuls of the previous sc,
    then the second score pair, so the PE never parks ahead of runnable
    work. Empirically the phase is bounded by PSUM port traffic
    (score-write + exp-read + AV-accumulate), ~4.6us per sc step.
  - Weight/activation DMAs split across the SP and ACT HWDGE queues so
    the first exp only waits for ~8MB of bf16.

Measured: HW ~402us/iteration (paired For_i-loop slope), rel err 2.9e-3.
"""

import sys

sys.path.insert(0, "/opt/trn_rl_repo")

import numpy as np

import concourse.bass as bass  # noqa: E402,F401
import concourse.tile as tile  # noqa: E402
from concourse import bacc, mybir  # noqa: E402
from concourse.masks import make_identity  # noqa: E402

F32 = mybir.dt.float32
F32R = mybir.dt.float32r
BF16 = mybir.dt.bfloat16
AF = mybir.ActivationFunctionType

P = 128          # partitions
TOK = 1024       # query tokens per core
T2 = 1024        # kv sequence length
C = 1024         # embed dim
H = 16           # heads
D = 64           # head dim
NCH = C // P     # 8 channel chunks
NS = T2 // P     # 8 kv-position chunks
TN = 512         # matmul moving-dim tile
SCALE = 1.0 / np.sqrt(D)

N_CORES = 8
B_FULL, T_FULL = 4, 2048

WNAMES = ("Wq", "Wk", "Wv", "Wo")


def build_program(loop_iters=None, phase="full"):
    """loop_iters: if set, wrap the body in a For_i hardware loop (timing).
    phase: "pre" (through K0/Q0/V01), "attn" (no out-proj), "full"."""
    nc = bacc.Bacc("TRN2", target_bir_lowering=False, debug=False,
                   num_devices=N_CORES)

    aps = {}
    aps["xs"] = nc.dram_tensor("xs", [TOK, C], BF16,
                               kind="ExternalInput").ap()
    aps["encs"] = nc.dram_tensor("encs", [T2, C], BF16,
                                 kind="ExternalInput").ap()
    for w in WNAMES:
        aps[w] = nc.dram_tensor(w, [C, C], BF16, kind="ExternalInput").ap()
    for b in ("bq", "bv", "bo"):
        aps[b] = nc.dram_tensor(b, [C], F32, kind="ExternalInput").ap()
    out = nc.dram_tensor("out", [TOK, C], F32, kind="ExternalOutput").ap()

    with tile.TileContext(nc) as tc:
        if loop_iters is not None:
            with tc.For_i(0, loop_iters, 1):
                _emit(nc, tc, aps, out, phase)
        else:
            _emit(nc, tc, aps, out, phase)

    nc.compile()
    return nc


def _row(ap):
    return ap.rearrange("(a c) -> a c", a=1)


def _emit(nc, tc, aps, out, phase="full"):
    from contextlib import ExitStack

    with ExitStack() as S:
        pConst = S.enter_context(tc.tile_pool(name="pConst", bufs=1))
        pW = {w: S.enter_context(tc.tile_pool(name=f"p{w}", bufs=NCH))
              for w in WNAMES}
        pXT = S.enter_context(tc.tile_pool(name="pXT", bufs=2))
        pPanel = S.enter_context(tc.tile_pool(name="pPanel", bufs=4))
        pK = S.enter_context(tc.tile_pool(name="pK", bufs=NCH))
        pQ = S.enter_context(tc.tile_pool(name="pQ", bufs=NCH))
        pV = S.enter_context(tc.tile_pool(name="pV", bufs=NS))
        pY = S.enter_context(tc.tile_pool(name="pY", bufs=NCH))

        psMM = S.enter_context(tc.tile_pool(name="psMM", bufs=4, space="PSUM"))

        # ---- constants ----
        idf = pConst.tile([P, P], F32, tag="idf", name="idf")
        make_identity(nc, idf)
        idb = pConst.tile([P, P], BF16, tag="idb", name="idb")
        nc.vector.tensor_copy(idb, idf)

        brow = pConst.tile([1, C], F32, tag="brow", name="brow_bq")
        nc.sync.dma_start(out=brow, in_=_row(aps["bq"]))
        bv_row = pConst.tile([1, C], F32, tag="bvr", name="bv_row")
        nc.sync.dma_start(out=bv_row, in_=_row(aps["bv"]))

        # ---- weights (already bf16 in DRAM; host pre-casts). Separate
        # pools per weight so no DMA WAR-blocks a queue behind late
        # projection matmuls (strict FIFO engine queues).
        bvb = pConst.tile([P, C], F32, tag="bvb", name="bvb")
        nc.gpsimd.partition_broadcast(bvb, bv_row)
        wp = {}

        # ---- bq as per-partition column: transpose [1,128] slices ----
        bcolT = pConst.tile([P, NCH], F32, tag="bcolT", name="bcolT")
        psB = psMM.tile([P, TN], F32, tag="mm", name="psB")
        for co in range(NCH):
            nc.tensor.transpose(
                psB[:, co:co + 1],
                brow[:, co * P:(co + 1) * P], idf[0:1, 0:1])
        nc.vector.tensor_copy(bcolT, psB[:, 0:NCH])

        # ---- transposed activations (bf16), [c-chunk, tokens] blocks ----
        encT = pXT.tile([P, NCH * T2], BF16, tag="xt", name="encT")
        xT = pXT.tile([P, NCH * TOK], BF16, tag="xt", name="xT")

        # enc panels + Wk ride SP; x panels + Wq/Wv/Wo ride ACT, so the
        # two HWDGE queues stream concurrently and the first exp only
        # waits for enc+x+Wk+Wq (~8MB of bf16).
        _transpose_in(nc, psMM, pPanel, aps["encs"], encT, idb, nc.sync)
        wp["Wk"] = _load_w_bf16(nc, pW["Wk"], aps["Wk"], "Wk", nc.sync)

        # ---- K proj chunk 0, x transposes, Q proj chunk 0 (so attention
        # can start early), then the first two V s-chunks ----
        kT = [None] * NCH
        qT = [None] * NCH
        vS = [None] * NS
        _transpose_in(nc, psMM, pPanel, aps["xs"], xT, idb, nc.scalar)
        wp["Wq"] = _load_w_bf16(nc, pW["Wq"], aps["Wq"], "Wq", nc.scalar)
        wp["Wv"] = _load_w_bf16(nc, pW["Wv"], aps["Wv"], "Wv", nc.scalar)
        wp["Wo"] = _load_w_bf16(nc, pW["Wo"], aps["Wo"], "Wo", nc.scalar)
        _proj_chunk(nc, psMM, pK, kT, wp["Wk"], encT, 0, None, "kT")
        _proj_chunk(nc, psMM, pQ, qT, wp["Wq"], xT, 0, bcolT, "qT")
        for sc in (0, 1):
            for st in _v_chunk_steps(nc, psMM, pV, vS, wp["Wv"], encT, bvb,
                                     sc):
                st()
        if phase == "pre":
            dummy = pY.tile([P, TOK], F32, tag="dummy", name="dummy", bufs=1)
            nc.vector.tensor_copy(dummy, kT[0])
            nc.sync.dma_start(out=out[0:P, :], in_=dummy)
            return

        # ---- attention: head pairs, exp-paced; next pair's K/Q proj
        # matmuls interleaved into the exp-wait bubbles ----
        yT = [None] * NCH
        with ExitStack() as S2:
            psACC = S2.enter_context(tc.tile_pool(name="psACC", bufs=4,
                                                  space="PSUM"))
            pP = S2.enter_context(tc.tile_pool(name="pP", bufs=8))
            pBc = S2.enter_context(tc.tile_pool(name="pBc", bufs=2))

            for ch in range(NCH):
                # next-pair projection work (and, for pair 0, the remaining
                # V s-chunks), emitted as interleavable closures that fill
                # the exp-wait bubbles
                proj_steps = []
                if ch == 0:
                    ksteps = _proj_chunk_steps(
                        nc, psMM, pK, kT, wp["Wk"], encT, 1, None, "kT")
                    qsteps = _proj_chunk_steps(
                        nc, psMM, pQ, qT, wp["Wq"], xT, 1, bcolT, "qT")
                    vsteps = [
                        _v_chunk_steps(nc, psMM, pV, vS, wp["Wv"], encT,
                                       bvb, sc) for sc in range(2, NS)]
                    # order so vS[j] is emitted >=1 sc-window before AV
                    # needs it, and K1/Q1 land early
                    proj_steps = (vsteps[0] + ksteps + vsteps[1] + qsteps
                                  + [s for v in vsteps[2:] for s in v])
                elif ch + 1 < NCH:
                    proj_steps = (
                        _proj_chunk_steps(nc, psMM, pK, kT, wp["Wk"], encT,
                                          ch + 1, None, "kT")
                        + _proj_chunk_steps(nc, psMM, pQ, qT, wp["Wq"], xT,
                                            ch + 1, bcolT, "qT"))
                pi = 0

                ya = {}
                for hx in (0, 1):
                    for tn in range(2):
                        ya[hx, tn] = psACC.tile([P, TN], F32, tag="acc",
                                                name=f"ya{hx}{tn}")

                # Software-pipelined emission, hand-ordered for the strict
                # in-order engine FIFOs. Per sc (prev = sc-1's pexp tiles):
                #   sA0,sB0        row-group-disjoint pair, run concurrent
                #   AV x3 (prev)   fills the PE queue while exp_A0/exp_B0 run
                #   sB1,sA1        second disjoint pair (WARs on this sc's
                #                  exps have resolved by now)
                #   AV x1 (prev), proj steps   remaining fill
                # The 4 exps chain back-to-back on ACT; the PE never blocks
                # ahead of runnable work.
                def score(hx, tn, sc):
                    ro = hx * D
                    ps = psMM.tile([P, TN], F32, tag="mm", name="psS")
                    nc.tensor.matmul(
                        ps,
                        kT[ch][ro:ro + D, sc * P:(sc + 1) * P],
                        qT[ch][ro:ro + D, tn * TN:(tn + 1) * TN],
                        start=True, stop=True,
                    )
                    pexp = pP.tile([P, TN], BF16, tag="p", name="pexp")
                    nc.scalar.activation(pexp, ps, AF.Exp,
                                         scale=float(SCALE))
                    return (hx, tn, pexp)

                def av(item, psc):
                    hx, tn, pexp = item
                    h = 2 * ch + hx
                    nc.tensor.matmul(
                        ya[hx, tn][0:D + 1, :],
                        vS[psc][:, h * (D + 1):(h + 1) * (D + 1)],
                        pexp,
                        start=(psc == 0), stop=(psc == NS - 1),
                    )

                pend = []
                for sc in range(NS + 1):
                    if sc < NS:
                        cur = [score(0, 0, sc), score(1, 0, sc)]
                        for item in pend[:3]:
                            av(item, sc - 1)
                        cur.append(score(1, 1, sc))
                        cur.append(score(0, 1, sc))
                        for item in pend[3:]:
                            av(item, sc - 1)
                        want = -(-len(proj_steps) * (sc + 1) // NS)  # ceil
                        while pi < want:
                            proj_steps[pi]()
                            pi += 1
                        pend = cur
                    else:
                        for item in pend:
                            av(item, sc - 1)
                while pi < len(proj_steps):
                    proj_steps[pi]()
                    pi += 1

                # normalize: y = ya[0:64] * (1 / ya[64]) -> yT (bf16)
                yT_ch = pY.tile([P, TOK], BF16, tag="yT", name=f"yT{ch}")
                yT[ch] = yT_ch
                for hx in (0, 1):
                    ro = hx * D
                    bcsb = pBc.tile([D, TOK], F32, tag="bc", name="bcsb")
                    for tn in range(2):
                        nc.vector.reciprocal(
                            bcsb[0:1, tn * TN:(tn + 1) * TN],
                            ya[hx, tn][D:D + 1, :])
                    nc.gpsimd.partition_broadcast(bcsb, bcsb[0:1, :])
                    for tn in range(2):
                        tsl = slice(tn * TN, (tn + 1) * TN)
                        nc.vector.tensor_mul(yT_ch[ro:ro + D, tsl],
                                             ya[hx, tn][0:D, :],
                                             bcsb[:, tsl])

        if phase == "attn":
            for cc in range(NCH):
                o = pY.tile([P, TOK], F32, tag="dummy", name="dm2", bufs=1)
                nc.vector.tensor_copy(o, yT[cc])
                nc.sync.dma_start(out=out[cc * P:(cc + 1) * P, :], in_=o)
            return

        # ---- output projection ----
        with ExitStack() as S3:
            psOut = S3.enter_context(tc.tile_pool(name="psOut", bufs=2,
                                                  space="PSUM"))
            pO = S3.enter_context(tc.tile_pool(name="pO", bufs=2))
            pOb = S3.enter_context(tc.tile_pool(name="pOb", bufs=1))
            bo_row = pOb.tile([1, C], F32, tag="bor", name="bo_row")
            nc.sync.dma_start(out=bo_row, in_=_row(aps["bo"]))
            bob = pOb.tile([P, C], F32, tag="bob", name="bob")
            nc.gpsimd.partition_broadcast(bob, bo_row)
            for tp in range(TOK // P):
                psO = psOut.tile([P, C], F32, tag="o", name="psO")
                for cc in range(NCH):
                    for nn in range(2):
                        nc.tensor.matmul(
                            psO[:, nn * TN:(nn + 1) * TN],
                            yT[cc][:, tp * P:(tp + 1) * P],
                            wp["Wo"][cc][:, nn * TN:(nn + 1) * TN],
                            start=(cc == 0), stop=(cc == NCH - 1),
                        )
                o_sb = pO.tile([P, C], F32, tag="osb", name="o_sb")
                nc.vector.tensor_add(o_sb, psO, bob)
                oq = (nc.sync, nc.scalar)[tp % 2]
                oq.dma_start(out=out[tp * P:(tp + 1) * P, :], in_=o_sb)


def _load_w_bf16(nc, pool, W, wname, queue=None):
    """Load bf16 weight [C, C] as NCH row-panels [128, C] (HWDGE DMA)."""
    panels = [None] * NCH
    q = queue if queue is not None else nc.scalar
    for kc in range(NCH):
        panels[kc] = pool.tile([P, C], BF16, tag="W", name=f"{wname}{kc}")
        q.dma_start(out=panels[kc], in_=W[kc * P:(kc + 1) * P, :])
    return panels


def _transpose_in(nc, psMM, pPanel, src, dstT, idb, queue):
    """DRAM bf16 [rows, C] -> dstT bf16 [128, NCH*rows] (chunk-major
    blocks). One [128,1024] bf16 PSUM tile (1 bank) per panel, one 2x-mode
    DVE copy."""
    rows = src.shape[0]
    d3 = dstT.rearrange("p (c t) -> p c t", c=NCH)
    for rp in range(rows // P):
        panel = pPanel.tile([P, C], BF16, tag="panel", name="panel")
        queue.dma_start(out=panel, in_=src[rp * P:(rp + 1) * P, :])
        ps = psMM.tile([P, C], BF16, tag="mm", name="psT")
        for cc in range(NCH):
            nc.tensor.transpose(
                ps[:, cc * P:(cc + 1) * P],
                panel[:, cc * P:(cc + 1) * P], idb)
        nc.vector.tensor_copy(
            d3[:, :, rp * P:(rp + 1) * P],
            ps.rearrange("p (c t) -> p c t", c=NCH))


def _v_chunk_steps(nc, psMM, pV, vS, wv_p, encT, bvb, sc):
    """V-proj s-chunk sc -> vS[sc] = [v+bv | 1] 65-col blocks per head,
    as 2 interleavable closures (one per c_out half)."""
    vS[sc] = pV.tile([P, H * (D + 1)], BF16, tag="v", name=f"vS{sc}")
    v3 = vS[sc].rearrange("p (b c) -> p b c", b=H)
    nc.vector.memset(v3[:, :, D:D + 1], 1.0)

    steps = []
    for nn in range(2):
        st = {}

        def mk_mm(cci, nn=nn, st=st):
            def go():
                if cci == 0:
                    st["ps"] = psMM.tile([P, TN], F32, tag="mm", name="psV")
                for cc in (2 * cci, 2 * cci + 1):
                    nc.tensor.matmul(
                        st["ps"],
                        encT[:, cc * T2 + sc * P: cc * T2 + (sc + 1) * P],
                        wv_p[cc][:, nn * TN:(nn + 1) * TN],
                        start=(cc == 0), stop=(cc == NCH - 1),
                    )
            return go

        def fin(nn=nn, st=st):
            nc.vector.tensor_add(
                v3[:, nn * 8:(nn + 1) * 8, 0:D],
                st["ps"].rearrange("p (b c) -> p b c", b=8),
                bvb[:, nn * TN:(nn + 1) * TN].rearrange(
                    "p (b c) -> p b c", b=8))
        steps += [mk_mm(cci) for cci in range(4)] + [fin]
    return steps


def _proj_chunk_steps(nc, psMM, pool, dst_list, w_p, actT, co, bcol, nm):
    """Projection chunk co as one interleavable closure. kc-outer /
    tn-inner so consecutive matmuls share the stationary weight slice
    (one LDWEIGHTS per kc instead of two)."""
    dst = pool.tile([P, TOK if nm == "qT" else T2], BF16, tag=nm,
                    name=f"{nm}{co}")
    dst_list[co] = dst
    steps = []
    for tn in range(2):
        st = {}

        def mk_mm(kci, tn=tn, st=st):
            def go():
                if kci == 0:
                    st["ps"] = psMM.tile([P, TN], F32, tag="mm",
                                         name=f"ps{nm}")
                for kc in (2 * kci, 2 * kci + 1):
                    nc.tensor.matmul(
                        st["ps"],
                        w_p[kc][:, co * P:(co + 1) * P],
                        actT[:, kc * TOK + tn * TN:
                             kc * TOK + (tn + 1) * TN],
                        start=(kc == 0), stop=(kc == NCH - 1),
                    )
            return go

        def fin(tn=tn, st=st):
            if bcol is None:
                nc.vector.tensor_copy(dst[:, tn * TN:(tn + 1) * TN],
                                      st["ps"])
            else:
                nc.vector.tensor_scalar_add(dst[:, tn * TN:(tn + 1) * TN],
                                            st["ps"], bcol[:, co:co + 1])
        steps += [mk_mm(kci) for kci in range(4)] + [fin]
    return steps


def _proj_chunk(nc, psMM, pool, dst_list, w_p, actT, co, bcol, nm):
    for step in _proj_chunk_steps(nc, psMM, pool, dst_list, w_p, actT, co,
                                  bcol, nm):
        step()


_CACHED = None


def _get_program():
    global _CACHED
    if _CACHED is None:
        _CACHED = build_program()
    return _CACHED


def make_in_maps(inputs):
    """Per-core input dicts (host casts activations/weights to bf16)."""
    import ml_dtypes
    bf = ml_dtypes.bfloat16
    x = np.ascontiguousarray(np.asarray(inputs["x"], np.float32)).astype(bf)
    enc_x = np.ascontiguousarray(
        np.asarray(inputs["enc_x"], np.float32)).astype(bf)
    weights = {k: np.ascontiguousarray(np.asarray(inputs[k], np.float32))
               for k in ("bq", "bv", "bo")}
    for k in ("Wq", "Wk", "Wv", "Wo"):
        weights[k] = np.ascontiguousarray(
            np.asarray(inputs[k], np.float32)).astype(bf)

    B, T, Cx = x.shape
    assert (B, T, Cx) == (B_FULL, T_FULL, C), (B, T, Cx)
    half = T // 2
    in_maps = []
    for core in range(N_CORES):
        b, th = core // 2, core % 2
        m = {"xs": np.ascontiguousarray(x[b, th * half:(th + 1) * half, :]),
             "encs": np.ascontiguousarray(enc_x[b])}
        m.update(weights)
        in_maps.append(m)
    return in_maps


def kernel(**inputs):
    in_maps = make_in_maps(inputs)
    nc = _get_program()

    from concourse.bass_utils import run_bass_kernel_spmd
    res = None
    last_err = None
    for _attempt in range(3):
        try:
            res = run_bass_kernel_spmd(nc, in_maps,
                                       core_ids=list(range(N_CORES)))
            break
        except Exception as e:  # transient NRT/axon failures: retry
            last_err = e
    if res is None:
        raise last_err

    half = T_FULL // 2
    outp = np.empty((B_FULL, T_FULL, C), dtype=np.float32)
    for core in range(N_CORES):
        b, th = core // 2, core % 2
        outp[b, th * half:(th + 1) * half, :] = res.results[core]["out"]
    return outp


if __name__ == "__main__":
    prog = build_program()
    n_inst = sum(len(blk.instructions) for fn in prog.m.functions
                 for blk in fn.blocks)
    print("built OK; instructions:", n_inst)
